# revision 1
# baseline (speedup 1.0000x reference)
"""MoE gate routing (softmax top-2 over 64 experts) on 8 Trainium2 NeuronCores.

Data-parallel over the token dim: each core handles 2048 tokens.
Host pre-transposes inp -> xT so the device sees [d, t] layout (contiguous
1 MiB DMAs, contraction dim on partitions). Weight is pre-swizzled so each
k-chunk's lhsT is a contiguous [128, 64] slice.

Per core:
  logitsT[e, t] = sum_k  W.T[k*128:(k+1)*128, :].T @ xT[k*128:(k+1)*128, t]
  (PSUM accumulation, 4 banks of [64 experts, 512 tokens])
  + bias (ACT, per-partition) -> PE transpose -> [128 tokens, 64 experts]
  -> DVE max/max_index (top-8 sort) for top-2 idx/val
  -> softmax scores: s1 = 1/sum(exp(l - m)), s2 = exp(v2 - m)/sum.
"""

import os
import numpy as np

N_TOKENS = 16384
D_MODEL = 4096
NUM_EXPERT = 64
N_CORES = 8
TPC = N_TOKENS // N_CORES  # 2048 tokens per core
KCH = D_MODEL // 128       # 32 contraction chunks
TCH = TPC // 512           # 4 token chunks per core
NBLK = TPC // 128          # 16 blocks of 128 tokens per core

_PROG = None
LAST_RESULT = None


def _build_program():
    from contextlib import ExitStack

    import concourse.bass as bass  # noqa: F401
    import concourse.tile as tile
    from concourse import bacc, mybir
    from concourse.masks import make_identity

    f32 = mybir.dt.float32
    u32 = mybir.dt.uint32
    AF = mybir.ActivationFunctionType

    nc = bacc.Bacc(
        "TRN2",
        target_bir_lowering=False,
        debug=False,
        num_devices=N_CORES,
    )
    xT = nc.dram_tensor("xT", [D_MODEL, TPC], f32, kind="ExternalInput").ap()
    wt = nc.dram_tensor("wt", [128, KCH * NUM_EXPERT], f32, kind="ExternalInput").ap()
    bias = nc.dram_tensor("bias", [NUM_EXPERT, 1], f32, kind="ExternalInput").ap()
    out_idx = nc.dram_tensor("out_idx", [128, NBLK * 2], u32, kind="ExternalOutput").ap()
    out_scr = nc.dram_tensor("out_scr", [128, NBLK * 2], f32, kind="ExternalOutput").ap()

    with ExitStack() as ctx:
        tc = ctx.enter_context(tile.TileContext(nc))
        const = ctx.enter_context(tc.tile_pool(name="const", bufs=1))
        xpool = ctx.enter_context(tc.tile_pool(name="x", bufs=3))
        epool = ctx.enter_context(tc.tile_pool(name="epi", bufs=2))
        acc_pool = ctx.enter_context(tc.tile_pool(name="acc", bufs=1, space="PSUM"))
        tp_pool = ctx.enter_context(tc.tile_pool(name="tp", bufs=2, space="PSUM"))

        wt_sb = const.tile([128, KCH * NUM_EXPERT], f32, name="wt_sb", tag="wt_sb")
        nc.sync.dma_start(wt_sb[:], wt)
        b_sb = const.tile([NUM_EXPERT, 1], f32, name="b_sb", tag="b_sb")
        nc.sync.dma_start(b_sb[:], bias)
        ident = const.tile([NUM_EXPERT, NUM_EXPERT], f32, name="ident", tag="ident")
        make_identity(nc, ident[:])
        oidx_sb = const.tile([128, NBLK, 2], u32, name="oidx_sb", tag="oidx_sb")
        oscr_sb = const.tile([128, NBLK, 2], f32, name="oscr_sb", tag="oscr_sb")

        acc = [
            acc_pool.tile([NUM_EXPERT, 512], f32, name=f"acc{t}", tag=f"acc{t}")
            for t in range(TCH)
        ]

        for k in range(KCH):
            xt = xpool.tile([128, TPC], f32, name="xt", tag="xt")
            nc.sync.dma_start(xt[:], xT[k * 128:(k + 1) * 128, :])
            for t in range(TCH):
                nc.tensor.matmul(
                    acc[t][:],
                    wt_sb[:, k * NUM_EXPERT:(k + 1) * NUM_EXPERT],
                    xt[:, t * 512:(t + 1) * 512],
                    start=(k == 0),
                    stop=(k == KCH - 1),
                )

        for t in range(TCH):
            # logitsT chunk + bias (bias is per-partition = per-expert here)
            lt = epool.tile([NUM_EXPERT, 512], f32, name="lt", tag="lt")
            nc.scalar.activation(lt[:], acc[t][:], AF.Identity, bias=b_sb[:], scale=1.0)
            for j in range(4):
                blk = t * 4 + j
                pt = tp_pool.tile([128, NUM_EXPERT], f32, name="pt", tag="pt")
                nc.tensor.transpose(pt[:], lt[:, j * 128:(j + 1) * 128], ident[:])
                lg = epool.tile([128, NUM_EXPERT], f32, name="lg", tag="lg")
                nc.vector.tensor_copy(lg[:], pt[:])

                max8 = epool.tile([128, 8], f32, name="max8", tag="max8")
                idx8 = epool.tile([128, 8], u32, name="idx8", tag="idx8")
                nc.vector.max(max8[:], lg[:])
                nc.vector.max_index(idx8[:], max8[:], lg[:])

                negm = epool.tile([128, 1], f32, name="negm", tag="negm")
                nc.vector.tensor_scalar_mul(negm[:], max8[:, 0:1], -1.0)
                et = epool.tile([128, NUM_EXPERT], f32, name="et", tag="et")
                ssum = epool.tile([128, 1], f32, name="ssum", tag="ssum")
                nc.scalar.activation(
                    et[:], lg[:], AF.Exp, bias=negm[:], accum_out=ssum[:]
                )
                r = epool.tile([128, 1], f32, name="r", tag="r")
                nc.vector.reciprocal(r[:], ssum[:])
                e2 = epool.tile([128, 1], f32, name="e2", tag="e2")
                nc.scalar.activation(e2[:], max8[:, 1:2], AF.Exp, bias=negm[:])
                s2 = epool.tile([128, 1], f32, name="s2", tag="s2")
                nc.vector.tensor_tensor(s2[:], e2[:], r[:], mybir.AluOpType.mult)

                nc.vector.tensor_copy(oidx_sb[:, blk, :], idx8[:, 0:2])
                nc.vector.tensor_copy(oscr_sb[:, blk, 0:1], r[:])
                nc.vector.tensor_copy(oscr_sb[:, blk, 1:2], s2[:])

        nc.sync.dma_start(out_idx, oidx_sb.rearrange("p a b -> p (a b)"))
        nc.sync.dma_start(out_scr, oscr_sb.rearrange("p a b -> p (a b)"))

    nc.compile()
    return nc


def _get_program():
    global _PROG
    if _PROG is None:
        _PROG = _build_program()
    return _PROG


def _prep_inputs(inp, W, b):
    inp = np.asarray(inp, dtype=np.float32)
    W = np.asarray(W, dtype=np.float32)
    b = np.asarray(b, dtype=np.float32)
    xT = np.ascontiguousarray(inp.T)  # [D, N]
    # wt[p, k*64+e] = W[e, k*128+p]
    wt = np.ascontiguousarray(
        W.T.reshape(KCH, 128, NUM_EXPERT).transpose(1, 0, 2).reshape(128, -1)
    )
    b2 = np.ascontiguousarray(b.reshape(NUM_EXPERT, 1))
    in_maps = []
    for c in range(N_CORES):
        in_maps.append(
            {
                "xT": np.ascontiguousarray(xT[:, c * TPC:(c + 1) * TPC]),
                "wt": wt,
                "bias": b2,
            }
        )
    return in_maps


def kernel(inp, W, b):
    global LAST_RESULT
    from concourse import bass_utils

    nc = _get_program()
    in_maps = _prep_inputs(inp, W, b)
    trace = os.environ.get("KERNEL_PROFILE", "0") == "1"
    res = bass_utils.run_bass_kernel_spmd(
        nc, in_maps, core_ids=list(range(N_CORES)), trace=trace
    )
    LAST_RESULT = res

    idx_parts = []
    scr_parts = []
    for c in range(N_CORES):
        oi = np.asarray(res.results[c]["out_idx"])
        osc = np.asarray(res.results[c]["out_scr"])
        idx_parts.append(
            oi.reshape(128, NBLK, 2).transpose(1, 0, 2).reshape(TPC, 2).astype(np.int32)
        )
        scr_parts.append(
            osc.reshape(128, NBLK, 2).transpose(1, 0, 2).reshape(TPC, 2)
        )
    return np.concatenate(idx_parts), np.concatenate(scr_parts)


# revision 3
# speedup vs baseline: 1.2661x; 1.2661x over previous
"""MoE gate routing (softmax top-2 over 64 experts) on 8 Trainium2 NeuronCores.

Data-parallel over the token dim: each core handles 2048 tokens.
Host pre-transposes inp -> xT so the device sees [d, t] layout (contiguous
2 MiB DMAs, contraction dim on partitions). Weight is pre-swizzled so each
k-chunk's lhsT is a contiguous [128, 64] slice.

Per core:
  - fp32 matmuls run column-tiled: chunk pairs (2i, 2i+1) execute
    concurrently in the two M=64 halves of the 128x128 PE array,
    accumulating into PSUM [0:64] / [64:128] of a [128, 512] bank.
  - halves are folded with a PE matmul against a stacked identity
    [I64; I64], bias added on ACT, PE-transpose to [128 tok, 64 expert],
    then DVE max/max_index (top-8 sort) gives top-2; softmax scores via
    fused ACT exp+accumulate and DVE reciprocal.
  - tokens are processed in two halves of 1024 so the first half's
    epilogue overlaps the second half's DMA+matmul stream.
"""

import os
import numpy as np

N_TOKENS = 16384
D_MODEL = 4096
NUM_EXPERT = 64
N_CORES = 8
TPC = N_TOKENS // N_CORES  # 2048 tokens per core
KCH = D_MODEL // 128       # 32 contraction chunks
NBLK = TPC // 128          # 16 blocks of 128 tokens per core
NTR = 8                    # DMA transfers per token-half (4 k-chunks each)

_PROG = None
LAST_RESULT = None


def _build_program():
    from contextlib import ExitStack

    import concourse.bass as bass  # noqa: F401
    import concourse.tile as tile
    from concourse import bacc, mybir

    f32 = mybir.dt.float32
    u32 = mybir.dt.uint32
    AF = mybir.ActivationFunctionType

    nc = bacc.Bacc(
        "TRN2",
        target_bir_lowering=False,
        debug=False,
        num_devices=N_CORES,
    )
    xT = nc.dram_tensor("xT", [D_MODEL, TPC], f32, kind="ExternalInput").ap()
    wt = nc.dram_tensor("wt", [128, KCH * NUM_EXPERT], f32, kind="ExternalInput").ap()
    bias = nc.dram_tensor("bias", [NUM_EXPERT, 1], f32, kind="ExternalInput").ap()
    id2 = nc.dram_tensor("ident2", [128, NUM_EXPERT], f32, kind="ExternalInput").ap()
    out_idx = nc.dram_tensor("out_idx", [128, NBLK * 2], u32, kind="ExternalOutput").ap()
    out_scr = nc.dram_tensor("out_scr", [128, NBLK * 2], f32, kind="ExternalOutput").ap()

    with ExitStack() as ctx:
        tc = ctx.enter_context(tile.TileContext(nc))
        const = ctx.enter_context(tc.tile_pool(name="const", bufs=1))
        xpool = ctx.enter_context(tc.tile_pool(name="x", bufs=3))
        epool = ctx.enter_context(tc.tile_pool(name="epi", bufs=2))
        acc_pool = ctx.enter_context(tc.tile_pool(name="acc", bufs=1, space="PSUM"))
        fold_pool = ctx.enter_context(tc.tile_pool(name="fold", bufs=2, space="PSUM"))
        tp_pool = ctx.enter_context(tc.tile_pool(name="tp", bufs=2, space="PSUM"))

        wt_sb = const.tile([128, KCH * NUM_EXPERT], f32, name="wt_sb", tag="wt_sb")
        nc.scalar.dma_start(wt_sb[:], wt)
        b_sb = const.tile([NUM_EXPERT, 1], f32, name="b_sb", tag="b_sb")
        nc.scalar.dma_start(b_sb[:], bias)
        ident2 = const.tile([128, NUM_EXPERT], f32, name="ident2", tag="ident2")
        nc.scalar.dma_start(ident2[:], id2)
        oidx_sb = const.tile([128, NBLK, 2], u32, name="oidx_sb", tag="oidx_sb")
        oscr_sb = const.tile([128, NBLK, 2], f32, name="oscr_sb", tag="oscr_sb")

        acc = [
            acc_pool.tile([128, 512], f32, name=f"acc{t}", tag=f"acc{t}")
            for t in range(4)
        ]

        def epilogue(tglob):
            # acc[tglob] holds logitsT split across partition halves.
            ltab = epool.tile([128, 512], f32, name="ltab", tag="ltab")
            nc.vector.tensor_copy(ltab[:], acc[tglob][:])
            # fold halves: [I64; I64].T @ ltab = ltab[0:64] + ltab[64:128]
            fps = fold_pool.tile([NUM_EXPERT, 512], f32, name="fps", tag="fps")
            nc.tensor.matmul(fps[:], ident2[:], ltab[:], start=True, stop=True)
            lt = epool.tile([NUM_EXPERT, 512], f32, name="lt", tag="lt")
            nc.scalar.activation(lt[:], fps[:], AF.Identity, bias=b_sb[:], scale=1.0)
            for j in range(4):
                blk = tglob * 4 + j
                pt = tp_pool.tile([128, NUM_EXPERT], f32, name="pt", tag="pt")
                nc.tensor.transpose(
                    pt[:], lt[:, j * 128:(j + 1) * 128], ident2[0:NUM_EXPERT, :]
                )
                lg = epool.tile([128, NUM_EXPERT], f32, name="lg", tag="lg")
                nc.vector.tensor_copy(lg[:], pt[:])

                max8 = epool.tile([128, 8], f32, name="max8", tag="max8")
                idx8 = epool.tile([128, 8], u32, name="idx8", tag="idx8")
                nc.vector.max(max8[:], lg[:])
                nc.vector.max_index(idx8[:], max8[:], lg[:])

                negm = epool.tile([128, 1], f32, name="negm", tag="negm")
                nc.vector.tensor_scalar_mul(negm[:], max8[:, 0:1], -1.0)
                et = epool.tile([128, NUM_EXPERT], f32, name="et", tag="et")
                ssum = epool.tile([128, 1], f32, name="ssum", tag="ssum")
                nc.scalar.activation(
                    et[:], lg[:], AF.Exp, bias=negm[:], accum_out=ssum[:]
                )
                r = epool.tile([128, 1], f32, name="r", tag="r")
                nc.vector.reciprocal(r[:], ssum[:])
                e2 = epool.tile([128, 1], f32, name="e2", tag="e2")
                nc.scalar.activation(e2[:], max8[:, 1:2], AF.Exp, bias=negm[:])
                s2 = epool.tile([128, 1], f32, name="s2", tag="s2")
                nc.vector.tensor_tensor(s2[:], e2[:], r[:], mybir.AluOpType.mult)

                nc.vector.tensor_copy(oidx_sb[:, blk, :], idx8[:, 0:2])
                nc.vector.tensor_copy(oscr_sb[:, blk, 0:1], r[:])
                nc.vector.tensor_copy(oscr_sb[:, blk, 1:2], s2[:])

        for half in range(2):
            for tr in range(NTR):
                # one transfer = 4 k-chunks x 1024 tokens = 2 MiB contiguous-ish
                xt = xpool.tile([128, 4, 1024], f32, name="xt", tag="xt")
                src = xT[
                    tr * 512:(tr + 1) * 512, half * 1024:(half + 1) * 1024
                ].rearrange("(i p) t -> p i t", p=128)
                eng = nc.sync if tr % 2 == 0 else nc.scalar
                eng.dma_start(xt[:], src)
                for t2 in range(2):
                    tglob = half * 2 + t2
                    for i2 in range(2):
                        kA = tr * 4 + 2 * i2
                        kB = kA + 1
                        first = tr == 0 and i2 == 0
                        last = tr == NTR - 1 and i2 == 1
                        nc.tensor.matmul(
                            acc[tglob][0:NUM_EXPERT, :],
                            wt_sb[:, kA * NUM_EXPERT:(kA + 1) * NUM_EXPERT],
                            xt[:, 2 * i2, t2 * 512:(t2 + 1) * 512],
                            start=first,
                            stop=last,
                            skip_group_check=True,
                        )
                        nc.tensor.matmul(
                            acc[tglob][NUM_EXPERT:128, :],
                            wt_sb[:, kB * NUM_EXPERT:(kB + 1) * NUM_EXPERT],
                            xt[:, 2 * i2 + 1, t2 * 512:(t2 + 1) * 512],
                            start=first,
                            stop=last,
                            skip_group_check=True,
                        )
            epilogue(half * 2 + 0)
            epilogue(half * 2 + 1)

        nc.sync.dma_start(out_idx, oidx_sb.rearrange("p a b -> p (a b)"))
        nc.sync.dma_start(out_scr, oscr_sb.rearrange("p a b -> p (a b)"))

    nc.compile()
    return nc


def _get_program():
    global _PROG
    if _PROG is None:
        _PROG = _build_program()
    return _PROG


def _prep_inputs(inp, W, b):
    inp = np.asarray(inp, dtype=np.float32)
    W = np.asarray(W, dtype=np.float32)
    b = np.asarray(b, dtype=np.float32)
    xT = np.ascontiguousarray(inp.T)  # [D, N]
    # wt[p, k*64+e] = W[e, k*128+p]
    wt = np.ascontiguousarray(
        W.T.reshape(KCH, 128, NUM_EXPERT).transpose(1, 0, 2).reshape(128, -1)
    )
    b2 = np.ascontiguousarray(b.reshape(NUM_EXPERT, 1))
    eye = np.eye(NUM_EXPERT, dtype=np.float32)
    id2 = np.ascontiguousarray(np.concatenate([eye, eye], axis=0))
    in_maps = []
    for c in range(N_CORES):
        in_maps.append(
            {
                "xT": np.ascontiguousarray(xT[:, c * TPC:(c + 1) * TPC]),
                "wt": wt,
                "bias": b2,
                "ident2": id2,
            }
        )
    return in_maps


def kernel(inp, W, b):
    global LAST_RESULT
    from concourse import bass_utils

    nc = _get_program()
    in_maps = _prep_inputs(inp, W, b)
    trace = os.environ.get("KERNEL_PROFILE", "0") == "1"
    res = bass_utils.run_bass_kernel_spmd(
        nc, in_maps, core_ids=list(range(N_CORES)), trace=trace
    )
    LAST_RESULT = res

    idx_parts = []
    scr_parts = []
    for c in range(N_CORES):
        oi = np.asarray(res.results[c]["out_idx"])
        osc = np.asarray(res.results[c]["out_scr"])
        idx_parts.append(
            oi.reshape(128, NBLK, 2).transpose(1, 0, 2).reshape(TPC, 2).astype(np.int32)
        )
        scr_parts.append(
            osc.reshape(128, NBLK, 2).transpose(1, 0, 2).reshape(TPC, 2)
        )
    return np.concatenate(idx_parts), np.concatenate(scr_parts)
